# revision 1
# baseline (speedup 1.0000x reference)
"""Levina-Bickel MLE intrinsic-dimension kernel for Trainium2 (8 NeuronCores).

Problem: X [B=4, N=8192, D=32] f32, k=16.
  d2[b,i,j] = |x_i - x_j|^2 ; per row take 16 smallest (incl. self), drop self,
  s_i = sum_j log(d_16/d_j), out[b] = 14*N / sum_i s_i.

Sharding: core c -> batch c//2, query rows (c%2)*4096 ..+4096, full X[b]
replicated as the matmul moving operand.

Per core:
  key[i,j] = q_i . x_j - 0.5*|x_j|^2  (ranking by largest key == smallest d2)
  computed as ONE K=99 bf16 matmul per block: partitions 0-31 hold q_hi/x_hi,
  32-33 the (1, -0.5|x|^2) hi/lo norm rows, 34-65 q_hi/x_lo, 66-97 q_lo/x_hi,
  and row 98 a constant C=1024 that shifts every key positive (so the rank-1..8
  mask in the merge can use multiply-by-0).  PE cost depends only on the moving
  dim, so the whole split is free -> near-fp32 accuracy at 1 bf16-matmul cost.
  Top-16 per row: DVE max8 per 1024-col block (8 blocks) -> 64 candidates,
  then max8 -> mask ranks 1-8 to 0 via (cands < v8)*cands -> max8 again for
  ranks 9-16 (match_replace is avoided: its MATCH_VALUE_LOAD stalls the DVE
  ~1us per call).  ACT computes L = ln((sq_i+2C) - 2*key') with per-partition
  bias and a fused free-dim accumulate, plus two Identity ops folding
  s'_i = 15*L_16 - sum(L), so the DVE stream never waits on ACT.
  Host sums the per-core [128, 32] partials and finishes the scalar math.
  Measured: ~325us HW exec on 8 cores, DVE-bound at ~100% duty (the max8
  stream at 1 elem/lane/cycle is this algorithm's floor).
"""

import sys

sys.path.insert(0, "/opt/trn_rl_repo")

import numpy as np
import ml_dtypes

import concourse.bass as bass  # noqa: F401  (registers bass types)
import concourse.bacc as bacc
import concourse.tile as tile
import concourse.mybir as mybir
from concourse.bass_utils import run_bass_kernel_spmd

BF16 = ml_dtypes.bfloat16
B, N, D, KNN = 4, 8192, 32, 16
NCORES = 8
ROWS_PER_CORE = B * N // NCORES      # 4096
TILES = ROWS_PER_CORE // 128         # 32
NBLK = 8
BLK = N // NBLK                      # block width for the per-block top-8
KEY_SHIFT = 1024.0  # d2 = (sq_i + 2C) - 2*key'

_compiled = None


def _build():
    nc = bacc.Bacc("TRN2", target_bir_lowering=False, debug=False)
    f32 = mybir.dt.float32
    bf16 = mybir.dt.bfloat16

    xt_d = nc.dram_tensor("xt", [128, N], bf16, kind="ExternalInput")
    qt_d = nc.dram_tensor("qt", [128, ROWS_PER_CORE], bf16, kind="ExternalInput")
    sq_d = nc.dram_tensor("sqq", [128, TILES], f32, kind="ExternalInput")
    out_d = nc.dram_tensor("acc_out", [128, TILES], f32, kind="ExternalOutput")

    with tile.TileContext(nc) as tc:
        with (
            tc.tile_pool(name="persist", bufs=1) as persist,
            tc.tile_pool(name="psum", bufs=4, space="PSUM") as psum_pool,
            tc.tile_pool(name="work", bufs=4) as work,
        ):
            xt = persist.tile([128, N], bf16)
            qt = persist.tile([128, ROWS_PER_CORE], bf16)
            sqq = persist.tile([128, TILES], f32)
            acc = persist.tile([128, TILES], f32)

            # tile 0's weights + first column block land first so the real
            # pipeline can start while the rest of the inputs stream in
            nc.sync.dma_start(qt[:, 0:128], qt_d.ap()[:, 0:128])
            nc.sync.dma_start(xt[:, 0:BLK], xt_d.ap()[:, 0:BLK])
            nc.sync.dma_start(qt[:, 128:], qt_d.ap()[:, 128:])
            for blk in range(1, NBLK):
                nc.sync.dma_start(xt[:, blk * BLK : (blk + 1) * BLK],
                                  xt_d.ap()[:, blk * BLK : (blk + 1) * BLK])
            nc.sync.dma_start(sqq[:], sq_d.ap()[:])


            def merge(t, cands):
                """Top-16 of the 64 block candidates + MLE fold for tile t."""
                sel = work.tile([128, 16], f32, tag="sel", name="sel")
                cands2 = work.tile([128, NBLK * 8], f32, tag="cands2", name="cands2")
                nc.vector.max(sel[:, 0:8], cands[:])
                # keys are > 0 (C shift), so masking ranks 1-8 to 0 drops them
                nc.vector.scalar_tensor_tensor(
                    cands2[:], cands[:], sel[:, 7:8], cands[:],
                    op0=mybir.AluOpType.is_lt, op1=mybir.AluOpType.mult,
                )
                nc.vector.max(sel[:, 8:16], cands2[:])

                logs = work.tile([128, KNN - 1], f32, tag="logs", name="logs")
                r = work.tile([128, 1], f32, tag="r", name="r")
                nc.scalar.activation(
                    logs[:], sel[:, 1:16], mybir.ActivationFunctionType.Ln,
                    bias=sqq[:, t : t + 1], scale=-2.0, accum_out=r[:],
                )
                # s' = 15*L_16 - sum(L), as two tiny ACT ops (Identity lives in
                # the same HW act table as Ln) so the DVE stream never waits.
                t15 = work.tile([128, 1], f32, tag="t15", name="t15")
                nc.scalar.activation(
                    t15[:], logs[:, KNN - 2 : KNN - 1],
                    mybir.ActivationFunctionType.Identity, scale=float(KNN - 1),
                )
                nc.scalar.activation(
                    acc[:, t : t + 1], r[:],
                    mybir.ActivationFunctionType.Identity, bias=t15[:], scale=-1.0,
                )

            # Software-pipelined: tile t's merge is emitted after tile t+1's
            # block max8s, so its dependencies are ~9us stale when the DVE
            # reaches it and the PE gets slack to run ahead.
            pending = None
            for t in range(TILES):
                w = qt[:, t * 128 : (t + 1) * 128]
                cands = work.tile([128, NBLK * 8], f32, tag="cands", name="cands")
                for blk in range(NBLK):
                    ps = psum_pool.tile([128, BLK], f32, tag="ps", name="ps")
                    for h in range(BLK // 512):
                        c0 = blk * BLK + h * 512
                        o = ps[:, h * 512 : (h + 1) * 512]
                        x = xt[:, c0 : c0 + 512]
                        nc.tensor.matmul(o, w[0:99, :], x[0:99, :],
                                         start=True, stop=True)
                    nc.vector.max(cands[:, blk * 8 : (blk + 1) * 8], ps[:])
                if pending is not None:
                    merge(*pending)
                pending = (t, cands)
            merge(*pending)

            nc.sync.dma_start(out_d.ap()[:], acc[:])

    nc.compile()
    return nc


def get_compiled():
    global _compiled
    if _compiled is None:
        _compiled = _build()
    return _compiled


def _split(a):
    hi = a.astype(BF16)
    lo = (a - hi.astype(np.float32)).astype(BF16)
    return hi, lo


def prep_inputs(X):
    """X [B, N, D] f32 -> per-core input maps + per-query |q|^2 table."""
    in_maps = []
    for c in range(NCORES):
        b, h = c // 2, c % 2
        Xb = np.ascontiguousarray(X[b])                       # [N, D] f32
        sqx = (Xb.astype(np.float64) ** 2).sum(1)             # [N] f64
        x33 = (-0.5 * sqx).astype(np.float32)
        Xhi, Xlo = _split(Xb)
        x33hi, x33lo = _split(x33)

        xt = np.zeros([128, N], BF16)
        xt[0:32] = Xhi.T
        xt[32] = x33hi
        xt[33] = x33lo
        xt[34:66] = Xlo.T
        xt[66:98] = Xhi.T
        xt[98] = BF16(KEY_SHIFT)

        Qb = Xb[h * ROWS_PER_CORE : (h + 1) * ROWS_PER_CORE]  # [4096, D]
        Qhi, Qlo = _split(Qb)
        qt = np.zeros([128, ROWS_PER_CORE], BF16)
        qt[0:32] = Qhi.T
        qt[32] = BF16(1.0)
        qt[33] = BF16(1.0)
        qt[34:66] = Qhi.T
        qt[66:98] = Qlo.T
        qt[98] = BF16(1.0)

        sq_core = (sqx[h * ROWS_PER_CORE : (h + 1) * ROWS_PER_CORE]
                   + 2.0 * KEY_SHIFT).astype(np.float32)
        sqq = np.ascontiguousarray(sq_core.reshape(TILES, 128).T)  # [128, TILES]

        in_maps.append({"xt": xt, "qt": qt, "sqq": sqq})
    return in_maps


def finish(acc_list):
    """acc_list: per-core [128, TILES] f32 of s'_i = 2*s_i. -> out [B] f32."""
    S = np.zeros(B, np.float64)
    for c, a in enumerate(acc_list):
        S[c // 2] += a.astype(np.float64).sum()
    # out_b = (k-2)*N / sum_i s_i  with  sum s_i = 0.5 * S_b
    return (2.0 * (KNN - 2) * N / S).astype(np.float32)


def kernel(X, k):
    assert int(k) == KNN
    X = np.asarray(X, dtype=np.float32)
    assert X.shape == (B, N, D)
    nc = get_compiled()
    in_maps = prep_inputs(X)
    # The axon tunnel occasionally throws a transient
    # NRT_EXEC_UNIT_UNRECOVERABLE on execute; a retry reliably recovers.
    last_err = None
    for _ in range(3):
        try:
            res = run_bass_kernel_spmd(nc, in_maps, list(range(NCORES)))
            acc_list = [res.results[c]["acc_out"] for c in range(NCORES)]
            return finish(acc_list)
        except Exception as e:  # noqa: BLE001 - device transients surface broadly
            last_err = e
    raise last_err



# revision 3
# speedup vs baseline: 1.1696x; 1.1696x over previous
"""Levina-Bickel MLE intrinsic-dimension kernel for Trainium2 (8 NeuronCores).

Problem: X [B=4, N=8192, D=32] f32, k=16.
  d2[b,i,j] = |x_i - x_j|^2 ; per row take 16 smallest (incl. self), drop self,
  s_i = sum_j log(d_16/d_j), out[b] = 14*N / sum_i s_i.

Sharding: core c -> batch c//2, query rows (c%2)*4096 ..+4096, full X[b]
replicated as the matmul moving operand.

v2 design (vs v1 = 8x max8-PSUM per tile, DVE-bound at ~10.4us/tile):
  The matmul emits y = SHIFT - d2 directly (bias rows folded into the K=100
  bf16 hi/lo contraction).  Per 128-row tile, 4 units of 2048 keys each:
  ACT casts the [128,2048] f32 PSUM tile to fp16 SBUF (1.97us, off the DVE),
  DVE folds pairwise max twice at 2x_1P (stride-1024 then stride-512, 4:1
  fold, 0.7+0.43us) and runs max8 on the two 256-wide halves (cov-1024
  blocks).  64 candidates/tile -> top-16 via max8 + is_lt-mask + max8.
  Selected y values land in a per-core [128, 32*16] buffer; ONE Ln pass +
  3D-AP tensor_reduce at the end compute s' = 15*L16 - sum(L) for all tiles
  (removes the per-tile ACT ops of v1).  Engine budget/tile: ACT 7.9us,
  DVE ~8.7us, PE ~7.3us (bf16 matmuls at 1.2GHz pipeline back-to-back).
  fp16 (not bf16) casts keep y quantization at 2^-5 => rel err ~2.4e-3
  (numpy-simulated), dominated by 4:1 fold collisions (4.4% of rows lose
  one neighbor to a same-slot collision).
"""

import sys

sys.path.insert(0, "/opt/trn_rl_repo")

import numpy as np
import ml_dtypes

import concourse.bass as bass  # noqa: F401  (registers bass types)
import concourse.bacc as bacc
import concourse.tile as tile
import concourse.mybir as mybir
from concourse.bass_utils import run_bass_kernel_spmd

BF16 = ml_dtypes.bfloat16
B, N, D, KNN = 4, 8192, 32, 16
NCORES = 8
ROWS_PER_CORE = B * N // NCORES      # 4096
TILES = ROWS_PER_CORE // 128         # 32
NUNIT = 4                            # 2048-wide units per tile
SHIFT = 64.0                         # y = SHIFT - d2; top-16 y stay > 0
CLAMP = SHIFT - 0.25                 # Ln input floor: d2_self -> 0.25
LN_SELF = float(np.log(0.25))        # ln of clamped self distance

_compiled = None


def _build():
    nc = bacc.Bacc("TRN2", target_bir_lowering=False, debug=False)
    f32 = mybir.dt.float32
    bf16 = mybir.dt.bfloat16
    fp16 = mybir.dt.float16

    xt_d = nc.dram_tensor("xt", [128, N], bf16, kind="ExternalInput")
    qt_d = nc.dram_tensor("qt", [128, ROWS_PER_CORE], bf16, kind="ExternalInput")
    out_d = nc.dram_tensor("acc_out", [128, TILES], f32, kind="ExternalOutput")

    with tile.TileContext(nc) as tc:
        with (
            tc.tile_pool(name="persist", bufs=1) as persist,
            tc.tile_pool(name="psum", bufs=2, space="PSUM") as psum_pool,
            tc.tile_pool(name="work", bufs=3) as work,
        ):
            xt = persist.tile([128, N], bf16)
            qt = persist.tile([128, ROWS_PER_CORE], bf16)
            selbuf = persist.tile([128, TILES * 16], f32)

            # tile 0's weights + first unit's keys land first so the pipeline
            # can start while the rest streams in
            nc.sync.dma_start(qt[:, 0:128], qt_d.ap()[:, 0:128])
            nc.sync.dma_start(xt[:, 0:2048], xt_d.ap()[:, 0:2048])
            nc.sync.dma_start(qt[:, 128:], qt_d.ap()[:, 128:])
            for u in range(1, NUNIT):
                nc.sync.dma_start(xt[:, u * 2048 : (u + 1) * 2048],
                                  xt_d.ap()[:, u * 2048 : (u + 1) * 2048])

            def merge(t, cands):
                """Top-16 of the 64 unit candidates -> selbuf[:, t*16:+16]."""
                s0 = t * 16
                nc.vector.max(selbuf[:, s0 : s0 + 8], cands[:])
                cands2 = work.tile([128, NUNIT * 16], f32, tag="c2", name="c2")
                # candidates are > 0 (top-16 y all positive), so masking the
                # first 8 ranks to 0 drops them from the second max8
                nc.vector.scalar_tensor_tensor(
                    cands2[:], cands[:], selbuf[:, s0 + 7 : s0 + 8], cands[:],
                    op0=mybir.AluOpType.is_lt, op1=mybir.AluOpType.mult,
                )
                nc.vector.max(selbuf[:, s0 + 8 : s0 + 16], cands2[:])

            pending = None
            for t in range(TILES):
                w = qt[:, t * 128 : (t + 1) * 128]
                cands = work.tile([128, NUNIT * 16], f32, tag="cands",
                                  name="cands")
                for u in range(NUNIT):
                    ps = psum_pool.tile([128, 2048], f32, tag="ps", name="ps")
                    for h in range(4):
                        c0 = u * 2048 + h * 512
                        nc.tensor.matmul(
                            ps[:, h * 512 : (h + 1) * 512],
                            w[0:100, :], xt[0:100, c0 : c0 + 512],
                            start=True, stop=True,
                        )
                    S = work.tile([128, 2048], fp16, tag="S", name="S")
                    nc.scalar.activation(
                        S[:], ps[:], mybir.ActivationFunctionType.Identity
                    )
                    C = work.tile([128, 1024], fp16, tag="C", name="C")
                    nc.vector.tensor_max(C[:], S[:, 0:1024], S[:, 1024:2048])
                    Dt = work.tile([128, 512], fp16, tag="D", name="D")
                    nc.vector.tensor_max(Dt[:], C[:, 0:512], C[:, 512:1024])
                    cb = u * 16
                    nc.vector.max(cands[:, cb : cb + 8], Dt[:, 0:256])
                    nc.vector.max(cands[:, cb + 8 : cb + 16], Dt[:, 256:512])
                    if u == 0 and pending is not None:
                        merge(*pending)
                        pending = None
                pending = (t, cands)
            merge(*pending)

            # batched tail: one Ln pass + per-tile reduction over the
            # [128, 32, 16] view of selbuf
            selc = persist.tile([128, TILES * 16], f32)
            nc.vector.tensor_scalar_min(selc[:], selbuf[:], CLAMP)
            logs = persist.tile([128, TILES * 16], f32)
            shiftc = persist.tile([128, 1], f32)
            nc.vector.memset(shiftc[:], SHIFT)
            nc.scalar.activation(
                logs[:], selc[:], mybir.ActivationFunctionType.Ln,
                bias=shiftc[:], scale=-1.0,
            )
            logs3 = logs[:].rearrange("p (g j) -> p g j", j=16)
            rsum = persist.tile([128, TILES], f32)
            nc.vector.tensor_reduce(
                rsum[:], logs3, axis=mybir.AxisListType.X,
                op=mybir.AluOpType.add,
            )
            t15 = persist.tile([128, TILES], f32)
            nc.vector.tensor_scalar_mul(t15[:], logs3[:, :, 15], float(KNN - 1))
            acc = persist.tile([128, TILES], f32)
            nc.vector.tensor_sub(acc[:], t15[:], rsum[:])
            nc.sync.dma_start(out_d.ap()[:], acc[:])

    nc.compile()
    return nc


def get_compiled():
    global _compiled
    if _compiled is None:
        _compiled = _build()
    return _compiled


def _split(a):
    hi = a.astype(BF16)
    lo = (a - hi.astype(np.float64)).astype(BF16)
    return hi, lo


def prep_inputs(X):
    """X [B, N, D] f32 -> per-core {xt, qt} maps (y = SHIFT - d2 layout)."""
    in_maps = []
    for c in range(NCORES):
        b, h = c // 2, c % 2
        Xb = np.ascontiguousarray(X[b])                       # [N, D] f32
        sqx = (Xb.astype(np.float64) ** 2).sum(1)             # [N] f64
        Xhi, Xlo = _split(Xb)
        nhi, nlo = _split(-sqx)

        xt = np.zeros([128, N], BF16)
        xt[0:32] = (Xhi.astype(np.float32) * 2.0).astype(BF16).T
        xt[32:64] = (Xlo.astype(np.float32) * 2.0).astype(BF16).T
        xt[64:96] = xt[0:32]
        xt[96] = nhi
        xt[97] = nlo
        xt[98] = BF16(1.0)
        xt[99] = BF16(1.0)

        Qb = Xb[h * ROWS_PER_CORE : (h + 1) * ROWS_PER_CORE]  # [4096, D]
        sqq = sqx[h * ROWS_PER_CORE : (h + 1) * ROWS_PER_CORE]
        Qhi, Qlo = _split(Qb)
        bhi, blo = _split(SHIFT - sqq)
        qt = np.zeros([128, ROWS_PER_CORE], BF16)
        qt[0:32] = Qhi.T
        qt[32:64] = Qhi.T
        qt[64:96] = Qlo.T
        qt[96] = BF16(1.0)
        qt[97] = BF16(1.0)
        qt[98] = bhi
        qt[99] = blo

        in_maps.append({"xt": xt, "qt": qt})
    return in_maps


def finish(acc_list):
    """acc_list: per-core [128, TILES] f32 of 15*L16 - sum_{j=0..15} L_j.

    2*s_i = acc_i + ln(0.25)  (rank-0 is the clamped self distance).
    out_b = 2*(k-2)*N / sum_i 2*s_i ... i.e. (k-2)*N / sum s_i.
    """
    S = np.zeros(B, np.float64)
    for c, a in enumerate(acc_list):
        S[c // 2] += (a.astype(np.float64) + LN_SELF).sum()
    return (2.0 * (KNN - 2) * N / S).astype(np.float32)


def kernel(X, k):
    assert int(k) == KNN
    X = np.asarray(X, dtype=np.float32)
    assert X.shape == (B, N, D)
    nc = get_compiled()
    in_maps = prep_inputs(X)
    # The axon tunnel occasionally throws a transient
    # NRT_EXEC_UNIT_UNRECOVERABLE on execute; a retry reliably recovers.
    last_err = None
    for _ in range(3):
        try:
            res = run_bass_kernel_spmd(nc, in_maps, list(range(NCORES)))
            acc_list = [res.results[c]["acc_out"] for c in range(NCORES)]
            return finish(acc_list)
        except Exception as e:  # noqa: BLE001 - device transients surface broadly
            last_err = e
    raise last_err
